# revision 31
# baseline (speedup 1.0000x reference)
"""Distributed Trainium2 Bass kernel for the AlignFormer block.

Sharding: sequence-parallel over 8 NeuronCores. Core i owns atom rows
[512*i, 512*(i+1)) and fasta rows [64*i, 64*(i+1)). Activations are kept in
transposed layout [D on partitions (4 chunks of 128), seq on free] so that
matmul chains need no transposes; layernorm statistics are computed with
ones-matmuls (partition-axis reduction on the TensorEngine).

Attention: scores are computed transposed (k on partitions, q on free), two
heads at a time (even head on PE rows 0-63, odd head on rows 64-127 so
LDWEIGHTS overlaps the other head's matmul); softmax skips the
max-subtraction (scores are O(5) for this problem, exp is exact) and the
denominator comes from a ones-augmented V column in the same PSUM
accumulation as att@V. Normalization is deferred: raw o and the per-head
denominators are collected, then one reciprocal + 4 indicator-matmuls
broadcast the scale factors.

The centroid gather runs against an all-gathered bf16 atom_trans; the fasta
branch is replicated on every core after a single AllGather of out_align.

Cross-core traffic: AllGather of K^T, V, atom_trans, out_align.
"""

from contextlib import ExitStack

import numpy as np
import ml_dtypes

import concourse.bass as bass
import concourse.mybir as mybir
from concourse import bacc, tile
from concourse.bass_utils import run_bass_kernel_spmd
from concourse.masks import make_identity

NCORES = 8
D = 512
NA = 4096
NF = 512
C = 4
KNB = 16
H = 8
DH = 64
FMID1 = 1024
FMID2 = 512
AOUT = 512
S = NA // NCORES   # 512 atom rows per core
F = NF // NCORES   # 64 fasta rows per core
P = 128
DCH = D // P       # 4 chunks of the model dim
BF = mybir.dt.bfloat16
F32 = mybir.dt.float32
I32 = mybir.dt.int32
SCALE = 1.0 / 8.0  # 1/sqrt(DH)
EPS = 1e-5
AF = mybir.ActivationFunctionType

bf16 = ml_dtypes.bfloat16


def build_nc():
    nc = bacc.Bacc("TRN2", target_bir_lowering=False, debug=False,
                   num_devices=NCORES)
    rg = [list(range(NCORES))]

    def mm(out, lhsT, rhs, start, stop):
        nc.tensor.matmul(out=out, lhsT=lhsT, rhs=rhs, start=start, stop=stop)

    # ---------------- external inputs ----------------
    x_sumT_bf = nc.dram_tensor("x_sumT_bf", [D, S], BF, kind="ExternalInput")
    x_fastaT_bf = nc.dram_tensor("x_fastaT_bf", [D, NF], BF,
                                 kind="ExternalInput")
    centpk = nc.dram_tensor("centpk", [P, 2 * KNB], I32, kind="ExternalInput")
    ind8 = nc.dram_tensor("ind8", [H, D], F32, kind="ExternalInput")

    a_in_w = nc.dram_tensor("a_in_w", [D, 3 * D], BF, kind="ExternalInput")
    a_out_w = nc.dram_tensor("a_out_w", [D, D], BF, kind="ExternalInput")
    fw1_w1 = nc.dram_tensor("fw1_w1", [D, FMID1], BF, kind="ExternalInput")
    fw1_w2 = nc.dram_tensor("fw1_w2", [FMID1, D], BF, kind="ExternalInput")
    fw2_w1 = nc.dram_tensor("fw2_w1", [D, FMID2], BF, kind="ExternalInput")
    fw2_w2 = nc.dram_tensor("fw2_w2", [FMID2, AOUT], BF, kind="ExternalInput")
    f_in_w = nc.dram_tensor("f_in_w", [D, 3 * D], BF, kind="ExternalInput")
    f_out_w = nc.dram_tensor("f_out_w", [D, D], BF, kind="ExternalInput")
    lrl_w1 = nc.dram_tensor("lrl_w1", [D, FMID2], BF, kind="ExternalInput")
    lrl_w2 = nc.dram_tensor("lrl_w2", [FMID2, AOUT], BF, kind="ExternalInput")

    # packed per-partition vectors ([128, nchunks])
    vec_specs = {
        "a_in_b_qk": 8, "a_out_b": 4, "an1_g": 4, "an1_b": 4,
        "fw1_b1": 8, "fw1_b2": 4, "an2_g": 4, "an2_b": 4,
        "fw2_b1": 4, "fw2_b2": 4, "f_in_b_qk": 8, "f_out_b": 4,
        "ans2_g": 4, "ans2_b": 4, "lrl_b1": 4, "lrl_b2": 4,
    }
    nvec = sum(vec_specs.values())
    vecpack = nc.dram_tensor("vecpack", [P, nvec], F32, kind="ExternalInput")
    vbias = nc.dram_tensor("vbias", [2, D], BF, kind="ExternalInput")

    out_atomT = nc.dram_tensor("out_atomT", [AOUT, S], F32,
                               kind="ExternalOutput")
    out_fastaT = nc.dram_tensor("out_fastaT", [AOUT, NF], F32,
                                kind="ExternalOutput")

    with tile.TileContext(nc) as tc, ExitStack() as ctx:
        dram = ctx.enter_context(tc.tile_pool(name="dram", bufs=1,
                                              space="DRAM"))
        const = ctx.enter_context(tc.tile_pool(name="const", bufs=1))
        persist = ctx.enter_context(tc.tile_pool(name="persist", bufs=1))
        wpool = ctx.enter_context(tc.tile_pool(name="wpool", bufs=2))
        expp = ctx.enter_context(tc.tile_pool(name="expp", bufs=3))
        vhp = ctx.enter_context(tc.tile_pool(name="vhp", bufs=2))
        psmm = ctx.enter_context(tc.tile_pool(name="psmm", bufs=2,
                                              space="PSUM"))
        pssc = ctx.enter_context(tc.tile_pool(name="pssc", bufs=2,
                                              space="PSUM"))
        psacc = ctx.enter_context(tc.tile_pool(name="psacc", bufs=2,
                                               space="PSUM"))

        def pt(shape, dtype, tag):
            return psmm.tile(shape, dtype, tag="mm", name=tag)

        # ---------------- DRAM bounce / collective buffers ----------------
        warm_in = dram.tile([1, P], BF)
        warm_out = dram.tile([NCORES, P], BF, addr_space="Shared")
        nc.gpsimd.collective_compute(
            "AllGather", mybir.AluOpType.bypass, ins=[warm_in[:]],
            outs=[warm_out[:]], replica_groups=rg)
        bounce_kv = dram.tile([2 * D, S], BF)
        kv_full = dram.tile([NCORES * 2 * D, S], BF, addr_space="Shared")
        bounce_atr = dram.tile([S, D], BF)
        atr_full = dram.tile([NA, D], BF, addr_space="Shared")
        bounce_oaa = dram.tile([F, D], F32)
        oaa_full = dram.tile([NF, D], F32, addr_space="Shared")

        # ---------------- constants ----------------
        ident = const.tile([P, P], F32)
        make_identity(nc, ident[:])
        ones_col = const.tile([P, 1], F32)
        nc.vector.memset(ones_col[:], 1.0)
        ones_row = const.tile([1, P], F32)
        nc.vector.memset(ones_row[:], 1.0)
        ones_row_bf = const.tile([1, P], BF)
        nc.vector.memset(ones_row_bf[:], 1.0)
        eps_tile = const.tile([1, 1], F32)
        nc.vector.memset(eps_tile[:], EPS)
        # warm the exp table early (one-time ~2.7us table load)
        dummy = const.tile([1, 1], F32)
        nc.scalar.activation(out=dummy[:], in_=ones_col[:1, :], func=AF.Exp)

        vp_sb = const.tile([P, nvec], F32)
        nc.sync.dma_start(out=vp_sb[:], in_=vecpack[:])
        VOFF = {}
        _off = 0
        for k, n in vec_specs.items():
            VOFF[k] = _off
            _off += n

        def vpsl(k, a, b):
            return vp_sb[:, VOFF[k] + (a):VOFF[k] + (b)]

        a_in_b_v_t = const.tile([1, D], BF)
        nc.sync.dma_start(out=a_in_b_v_t[:], in_=vbias[0:1, :])
        f_in_b_v_t = const.tile([1, D], BF)
        nc.sync.dma_start(out=f_in_b_v_t[:], in_=vbias[1:2, :])
        a_in_b_v_sb = a_in_b_v_t[:]
        f_in_b_v_sb = f_in_b_v_t[:]
        centpk_sb = const.tile([P, 2 * KNB], I32)
        nc.sync.dma_start(out=centpk_sb[:], in_=centpk[:])
        cent01_sb = centpk_sb[:, 0:KNB]
        cent23_sb = centpk_sb[:, KNB:2 * KNB]
        ind_sb = const.tile([H, D], F32)
        nc.sync.dma_start(out=ind_sb[:], in_=ind8[:])

        def load_w(ext, nch, width, tag):
            t = persist.tile([P, nch, width], BF, name=ext.name + "_sb",
                             tag=tag)
            nc.sync.dma_start(
                out=t[:], in_=ext[:].rearrange("(c p) n -> p c n", p=P))
            return t

        # only the QKV weight is needed before the collectives
        a_in_w_sb = load_w(a_in_w, DCH, 3 * D, "w_in")

        # ---------------- phase A: load pre-transposed atom_in ----------
        atom_inT_bf = persist.tile([P, DCH, S], BF, tag="bfA")
        nc.sync.dma_start(
            out=atom_inT_bf[:],
            in_=x_sumT_bf[:].rearrange("(c p) s -> p c s", p=P))

        # ---------------- phase B: QKV projections + allgathers ----------
        for m in range(DCH):  # K^T -> bounce -> AG (first, to overlap)
            ps = pt([P, S], F32, "ps_mm")
            for kc in range(DCH):
                mm(ps[:], a_in_w_sb[:, kc, D + m * P:D + (m + 1) * P],
                   atom_inT_bf[:, kc, :], kc == 0, kc == DCH - 1)
            stg = wpool.tile([P, S], BF, tag="stage", name="stg")
            nc.vector.tensor_scalar_add(
                out=stg[:], in0=ps[:], scalar1=vpsl("a_in_b_qk", 4 + m, 5 + m))
            nc.sync.dma_start(out=bounce_kv[m * P:(m + 1) * P, :], in_=stg[:])

        for st in range(4):  # V rows -> bounce -> AG
            ps = pt([P, D], F32, "ps_mm")
            for kc in range(DCH):
                mm(ps[:], atom_inT_bf[:, kc, st * P:(st + 1) * P],
                   a_in_w_sb[:, kc, 2 * D:3 * D], kc == 0, False)
            mm(ps[:], ones_row_bf[:], a_in_b_v_sb, False, True)
            stg = wpool.tile([P, S], BF, tag="stage", name="stg")
            nc.vector.tensor_copy(out=stg[:], in_=ps[:])
            nc.sync.dma_start(out=bounce_kv[D + st * P:D + (st + 1) * P, :],
                              in_=stg[:])
        nc.gpsimd.collective_compute(
            "AllGather", mybir.AluOpType.bypass, ins=[bounce_kv[:]],
            outs=[kv_full[:]], replica_groups=rg)

        qT = persist.tile([P, DCH, S], BF)   # Q^T stays local
        for m in range(DCH):
            ps = pt([P, S], F32, "ps_mm")
            for kc in range(DCH):
                mm(ps[:], a_in_w_sb[:, kc, m * P:(m + 1) * P],
                   atom_inT_bf[:, kc, :], kc == 0, kc == DCH - 1)
            nc.vector.tensor_scalar_add(
                out=qT[:, m, :], in0=ps[:],
                scalar1=vpsl("a_in_b_qk", m, m + 1))

        # ---------------- fasta pre-work (independent of the gather) -----
        f_in_w_sb = load_w(f_in_w, DCH, 3 * D, "w_in")
        fxT_bf = persist.tile([P, DCH, S], BF, tag="fxbf")
        nc.sync.dma_start(
            out=fxT_bf[:],
            in_=x_fastaT_bf[:].rearrange("(c p) s -> p c s", p=P))
        k1T = persist.tile([P, DCH, S], BF, tag="k1T")
        for m in range(DCH):
            ps = pt([P, S], F32, "ps_mm")
            for kc in range(DCH):
                mm(ps[:], f_in_w_sb[:, kc, D + m * P:D + (m + 1) * P],
                   fxT_bf[:, kc, :], kc == 0, kc == DCH - 1)
            nc.vector.tensor_scalar_add(
                out=k1T[:, m, :], in0=ps[:],
                scalar1=vpsl("f_in_b_qk", 4 + m, 5 + m))
        q1T = persist.tile([P, DCH, S], BF, tag="q1T")
        for m in range(DCH):
            ps = pt([P, S], F32, "ps_mm")
            for kc in range(DCH):
                mm(ps[:], f_in_w_sb[:, kc, m * P:(m + 1) * P],
                   fxT_bf[:, kc, :], kc == 0, kc == DCH - 1)
            nc.vector.tensor_scalar_add(
                out=q1T[:, m, :], in0=ps[:],
                scalar1=vpsl("f_in_b_qk", m, m + 1))
        v1r = persist.tile([P, 4, D], BF, tag="v1r")
        for st in range(4):
            ps = pt([P, D], F32, "ps_mm")
            for kc in range(DCH):
                mm(ps[:], fxT_bf[:, kc, st * P:(st + 1) * P],
                   f_in_w_sb[:, kc, 2 * D:3 * D], kc == 0, False)
            mm(ps[:], ones_row_bf[:], f_in_b_v_sb, False, True)
            nc.vector.tensor_copy(out=v1r[:, st, :], in_=ps[:])

        # remaining weights (loads overlap the collectives / attention)
        a_out_w_sb = load_w(a_out_w, DCH, D, "w_proj")
        fw1_w1_sb = load_w(fw1_w1, DCH, FMID1, "w_big1")
        fw1_w2_sb = load_w(fw1_w2, FMID1 // P, D, "w_big2")
        fw2_w1_sb = load_w(fw2_w1, DCH, FMID2, "w_fw2a")
        fw2_w2_sb = load_w(fw2_w2, FMID2 // P, AOUT, "w_fw2b")

        # ---------------- phase C/D: attention (head pairs) --------------
        ktb = []
        for b in range(NCORES):
            kt_t = persist.tile([P, DCH, S], BF, name=f"ktb{b}",
                                tag=f"ktb{b}")
            nc.sync.dma_start(
                out=kt_t[:],
                in_=kv_full[b * 2 * D:b * 2 * D + D, :].rearrange(
                    "(c p) s -> p c s", p=P))
            ktb.append(kt_t)
        v_full_r = kv_full[:].rearrange(
            "(b t st p) (hh j) -> t p b st hh j", t=2, st=4, p=P, j=DH)

        oTr = persist.tile([P, DCH, S], BF)       # raw (unnormalized) o^T
        stageA = wpool.tile([P, S], F32, tag="s01", bufs=1, name="stageA")
        stageB = wpool.tile([P, S], F32, tag="s23", bufs=1, name="stageB")
        for hp in range(4):
            vh0 = vhp.tile([P, 8, 4, DH + 1], BF, tag="vh", name="vh0")
            nc.gpsimd.memset(vh0[:, :, :, DH:DH + 1], 1.0)
            for b in range(NCORES):
                nc.sync.dma_start(out=vh0[:, b, :, 0:DH],
                                  in_=v_full_r[1, :, b, :, 2 * hp, :])
            vh1 = vhp.tile([P, 8, 4, DH + 1], BF, tag="vh", name="vh1")
            nc.gpsimd.memset(vh1[:, :, :, DH:DH + 1], 1.0)
            for b in range(NCORES):
                nc.sync.dma_start(out=vh1[:, b, :, 0:DH],
                                  in_=v_full_r[1, :, b, :, 2 * hp + 1, :])
            ps_o0 = psacc.tile([DH + 1, S], F32, tag="acc", name="ps_o0")
            ps_o1 = psacc.tile([DH + 1, S], F32, tag="acc", name="ps_o1")
            for kt in range(32):
                b, ktl = kt // 4, kt % 4
                ps_s = pssc.tile([P, 2 * S], F32, tag="sc", name="ps_s")
                mm(ps_s[:, 0:S],
                   ktb[b][0:DH, hp, ktl * P:(ktl + 1) * P],
                   qT[0:DH, hp, :], True, True)
                mm(ps_s[:, S:2 * S],
                   ktb[b][DH:P, hp, ktl * P:(ktl + 1) * P],
                   qT[DH:P, hp, :], True, True)
                et = expp.tile([P, 2 * S], BF, tag="expT", name="et")
                nc.scalar.activation(out=et[:], in_=ps_s[:], func=AF.Exp,
                                     scale=SCALE)
                mm(ps_o0[:], vh0[:, b, ktl, :], et[:, 0:S], kt == 0, kt == 31)
                mm(ps_o1[:], vh1[:, b, ktl, :], et[:, S:2 * S], kt == 0,
                   kt == 31)
            nc.vector.tensor_copy(out=oTr[0:DH, hp, :], in_=ps_o0[0:DH, :])
            nc.vector.tensor_copy(out=oTr[DH:P, hp, :], in_=ps_o1[0:DH, :])
            nc.vector.tensor_copy(out=stageA[32 * hp:32 * hp + 1, :],
                                  in_=ps_o0[DH:DH + 1, :])
            nc.vector.tensor_copy(out=stageB[32 * hp:32 * hp + 1, :],
                                  in_=ps_o1[DH:DH + 1, :])
        # deferred softmax normalization: oT = oTr * (1/osum) per (head, q)
        osum8 = persist.tile([H, S], F32, tag="osum8")
        for h in range(H):
            src = stageA if h % 2 == 0 else stageB
            nc.sync.dma_start(out=osum8[h:h + 1, :],
                              in_=src[32 * (h // 2):32 * (h // 2) + 1, :])
        orec = persist.tile([H, S], F32, tag="orec")
        nc.vector.reciprocal(out=orec[:], in_=osum8[:])
        oT = persist.tile([P, DCH, S], BF)
        for ch in range(DCH):
            ps_r = pt([P, S], F32, "ps_rbc")
            mm(ps_r[:], ind_sb[:, ch * P:(ch + 1) * P], orec[:], True, True)
            nc.vector.tensor_mul(out=oT[:, ch, :], in0=oTr[:, ch, :],
                                 in1=ps_r[:])

        # ---------------- layernorm helper (transposed layout) -----------
        def layernorm_T(pre, gk, bk, out_bf, out_f32=None):
            Sw = pre.shape[2]
            ps_mean = pt([1, Sw], F32, "ps_mean")
            ps_var = pt([1, Sw], F32, "ps_var")
            for m in range(DCH):
                mm(ps_mean[:], ones_col[:], pre[:, m, :], m == 0, m == DCH - 1)
                sq = wpool.tile([P, Sw], F32, tag="sq", bufs=1, name="sq")
                nc.scalar.activation(out=sq[:], in_=pre[:, m, :],
                                     func=AF.Square)
                mm(ps_var[:], ones_col[:], sq[:], m == 0, m == DCH - 1)
            mvec = wpool.tile([1, Sw], F32, tag="lnvec", bufs=3, name="mvec")
            nc.scalar.mul(out=mvec[:], in_=ps_mean[:], mul=1.0 / D)
            vvec = wpool.tile([1, Sw], F32, tag="lnvec", bufs=3, name="vvec")
            nc.scalar.mul(out=vvec[:], in_=ps_var[:], mul=1.0 / D)
            msq = wpool.tile([1, Sw], F32, tag="lnvec", bufs=3, name="msq")
            nc.vector.tensor_mul(out=msq[:], in0=mvec[:], in1=mvec[:])
            nc.vector.tensor_sub(out=vvec[:], in0=vvec[:], in1=msq[:])
            sd = wpool.tile([1, Sw], F32, tag="lnvec", bufs=3, name="sd")
            nc.scalar.activation(out=sd[:], in_=vvec[:], func=AF.Sqrt,
                                 bias=eps_tile[:])
            rstd = wpool.tile([1, Sw], F32, tag="lnvec", bufs=3, name="rstd")
            nc.vector.reciprocal(out=rstd[:], in_=sd[:])
            ps_mb = pt([P, Sw], F32, "ps_mb")
            mm(ps_mb[:], ones_row[:], mvec[:], True, True)
            ps_rb = pt([P, Sw], F32, "ps_rb")
            mm(ps_rb[:], ones_row[:], rstd[:], True, True)
            for m in range(DCH):
                t1 = wpool.tile([P, Sw], F32, tag="lnt1", bufs=1, name="lnt1")
                nc.vector.tensor_sub(out=t1[:], in0=pre[:, m, :], in1=ps_mb[:])
                nc.vector.tensor_mul(out=t1[:], in0=t1[:], in1=ps_rb[:])
                if out_f32 is not None:
                    nc.vector.tensor_scalar(
                        out=out_f32[:, m, :], in0=t1[:],
                        scalar1=vpsl(gk, m, m + 1), scalar2=vpsl(bk, m, m + 1),
                        op0=mybir.AluOpType.mult, op1=mybir.AluOpType.add)
                    nc.scalar.copy(out=out_bf[:, m, :], in_=out_f32[:, m, :])
                else:
                    nc.vector.tensor_scalar(
                        out=out_bf[:, m, :], in0=t1[:],
                        scalar1=vpsl(gk, m, m + 1), scalar2=vpsl(bk, m, m + 1),
                        op0=mybir.AluOpType.mult, op1=mybir.AluOpType.add)

        # ---------------- phase E: out-proj + residual + LN1 -------------
        h1 = persist.tile([P, DCH, S], F32, tag="h1")
        for m in range(DCH):
            ps = pt([P, S], F32, "ps_mm")
            for kc in range(DCH):
                mm(ps[:], a_out_w_sb[:, kc, m * P:(m + 1) * P],
                   oT[:, kc, :], kc == 0, kc == DCH - 1)
            t1 = wpool.tile([P, S], F32, tag="oproj", bufs=1, name="oproj")
            nc.scalar.activation(out=t1[:], in_=ps[:], func=AF.Identity,
                                 bias=vpsl("a_out_b", m, m + 1))
            nc.vector.tensor_add(out=h1[:, m, :], in0=t1[:],
                                 in1=atom_inT_bf[:, m, :])
        h1_bf = persist.tile([P, DCH, S], BF, tag="h1_bf")
        layernorm_T(h1, "an1_g", "an1_b", h1_bf, h1)

        # ---------------- phase F: FFN1 + atom_trans allgather -----------
        fmid = persist.tile([P, FMID1 // P, S], BF, tag="slotA")
        for m in range(FMID1 // P):
            ps = pt([P, S], F32, "ps_mm")
            for kc in range(DCH):
                mm(ps[:], fw1_w1_sb[:, kc, m * P:(m + 1) * P],
                   h1_bf[:, kc, :], kc == 0, kc == DCH - 1)
            nc.scalar.activation(out=fmid[:, m, :], in_=ps[:], func=AF.Relu,
                                 bias=vpsl("fw1_b1", m, m + 1))
        atrT = persist.tile([P, DCH, S], F32, tag="slotC")
        for m in range(DCH):
            ps = pt([P, S], F32, "ps_mm")
            for kc in range(FMID1 // P):
                mm(ps[:], fw1_w2_sb[:, kc, m * P:(m + 1) * P],
                   fmid[:, kc, :], kc == 0, kc == FMID1 // P - 1)
            nc.scalar.activation(out=atrT[:, m, :], in_=ps[:],
                                 func=AF.Identity,
                                 bias=vpsl("fw1_b2", m, m + 1))
        for st in range(4):  # back to rows (bf16) for the gather allgather
            arow = wpool.tile([P, D], BF, tag="arow", bufs=1, name="arow")
            for dc in range(DCH):
                pst = pt([P, P], F32, "ps_tr")
                nc.tensor.transpose(
                    out=pst[:], in_=atrT[:, dc, st * P:(st + 1) * P],
                    identity=ident[:])
                nc.vector.tensor_copy(out=arow[:, dc * P:(dc + 1) * P],
                                      in_=pst[:])
            nc.sync.dma_start(out=bounce_atr[st * P:(st + 1) * P, :],
                              in_=arow[:])
        nc.gpsimd.collective_compute(
            "AllGather", mybir.AluOpType.bypass, ins=[bounce_atr[:]],
            outs=[atr_full[:]], replica_groups=rg)

        # ---------------- phase G: LN2 + FFN2 -> atom_out ----------------
        h2pre = persist.tile([P, DCH, S], F32, tag="h2pre")
        for m in range(DCH):
            nc.vector.tensor_add(out=h2pre[:, m, :], in0=h1[:, m, :],
                                 in1=atrT[:, m, :])
        h2_bf = persist.tile([P, DCH, S], BF, tag="h2_bf")
        layernorm_T(h2pre, "an2_g", "an2_b", h2_bf)
        fmid2 = persist.tile([P, FMID2 // P, S], BF, tag="bfA")
        for m in range(FMID2 // P):
            ps = pt([P, S], F32, "ps_mm")
            for kc in range(DCH):
                mm(ps[:], fw2_w1_sb[:, kc, m * P:(m + 1) * P],
                   h2_bf[:, kc, :], kc == 0, kc == DCH - 1)
            nc.scalar.activation(out=fmid2[:, m, :], in_=ps[:], func=AF.Relu,
                                 bias=vpsl("fw2_b1", m, m + 1))
        for m in range(AOUT // P):
            ps = pt([P, S], F32, "ps_mm")
            for kc in range(FMID2 // P):
                mm(ps[:], fw2_w2_sb[:, kc, m * P:(m + 1) * P],
                   fmid2[:, kc, :], kc == 0, kc == FMID2 // P - 1)
            aout = wpool.tile([P, S], F32, tag="aout", bufs=1, name="aout")
            nc.scalar.activation(out=aout[:], in_=ps[:], func=AF.Identity,
                                 bias=vpsl("fw2_b2", m, m + 1))
            nc.sync.dma_start(out=out_atomT[m * P:(m + 1) * P, :], in_=aout[:])

        # ---------------- phase H: centroid gather + DVE max-accum ------
        gath0 = persist.tile([P, D], BF, tag="gath0")
        gath1 = persist.tile([P, D], BF, tag="gath1")
        for (gt, ct) in ((gath0, cent01_sb), (gath1, cent23_sb)):
            nc.vector.memset(gt[:], -3.0e38)
            for k in range(KNB):
                gb = wpool.tile([P, D], BF, tag="gbuf", bufs=2, name="gb")
                nc.gpsimd.indirect_dma_start(
                    out=gb[:], out_offset=None, in_=atr_full[:],
                    in_offset=bass.IndirectOffsetOnAxis(ap=ct[:, k:k + 1],
                                                        axis=0))
                nc.vector.tensor_tensor(out=gt[:], in0=gt[:], in1=gb[:],
                                        op=mybir.AluOpType.max)
        oaa = persist.tile([F, D], F32)
        s01 = wpool.tile([F, D], F32, tag="s01", bufs=1, name="s01")
        nc.vector.tensor_copy(out=s01[:], in_=gath0[F:P, :])
        nc.vector.tensor_add(out=s01[:], in0=s01[:], in1=gath0[0:F, :])
        s23 = wpool.tile([F, D], F32, tag="s23", bufs=1, name="s23")
        nc.vector.tensor_copy(out=s23[:], in_=gath1[F:P, :])
        nc.vector.tensor_add(out=s23[:], in0=s23[:], in1=gath1[0:F, :])
        nc.vector.tensor_add(out=oaa[:], in0=s01[:], in1=s23[:])
        nc.sync.dma_start(out=bounce_oaa[:], in_=oaa[:])
        nc.gpsimd.collective_compute(
            "AllGather", mybir.AluOpType.bypass, ins=[bounce_oaa[:]],
            outs=[oaa_full[:]], replica_groups=rg)

        # ---------------- phase I: fasta branch (replicated) -------------
        f_out_w_sb = load_w(f_out_w, DCH, D, "w_proj")
        lrl_w1_sb = load_w(lrl_w1, DCH, FMID2, "w_big1")
        lrl_w2_sb = load_w(lrl_w2, FMID2 // P, AOUT, "w_big2")
        # oaa part of fq: transpose gathered out_align and finish QKV
        oaaT_bf = persist.tile([P, DCH, S], BF, tag="oTr")
        for st in range(4):
            wa = wpool.tile([P, D], F32, tag="ld", bufs=2, name="wa")
            nc.sync.dma_start(out=wa[:],
                              in_=oaa_full[st * P:(st + 1) * P, :])
            for dc in range(DCH):
                pst = pt([P, P], F32, "ps_tr")
                nc.tensor.transpose(out=pst[:],
                                    in_=wa[:, dc * P:(dc + 1) * P],
                                    identity=ident[:])
                nc.scalar.copy(
                    out=oaaT_bf[:, dc, st * P:(st + 1) * P], in_=pst[:])
        fqT_bf = persist.tile([P, DCH, S], BF, tag="h2pre")
        nc.vector.tensor_add(out=fqT_bf[:], in0=fxT_bf[:], in1=oaaT_bf[:])
        for m in range(DCH):
            ps = pt([P, S], F32, "ps_mm")
            for kc in range(DCH):
                mm(ps[:], f_in_w_sb[:, kc, D + m * P:D + (m + 1) * P],
                   oaaT_bf[:, kc, :], kc == 0, kc == DCH - 1)
            nc.vector.tensor_add(out=k1T[:, m, :], in0=ps[:],
                                 in1=k1T[:, m, :])
        fkT = k1T
        for st in range(4):
            ps = pt([P, D], F32, "ps_mm")
            for kc in range(DCH):
                mm(ps[:], oaaT_bf[:, kc, st * P:(st + 1) * P],
                   f_in_w_sb[:, kc, 2 * D:3 * D], kc == 0, kc == DCH - 1)
            nc.vector.tensor_add(out=v1r[:, st, :], in0=ps[:],
                                 in1=v1r[:, st, :])
        fvrows = v1r
        for m in range(DCH):
            ps = pt([P, S], F32, "ps_mm")
            for kc in range(DCH):
                mm(ps[:], f_in_w_sb[:, kc, m * P:(m + 1) * P],
                   oaaT_bf[:, kc, :], kc == 0, kc == DCH - 1)
            nc.vector.tensor_add(out=q1T[:, m, :], in0=ps[:],
                                 in1=q1T[:, m, :])
        fqTq = q1T

        foTr = persist.tile([P, DCH, S], BF, tag="oTr")
        fstageA = wpool.tile([P, S], F32, tag="s01", bufs=1, name="fstageA")
        fstageB = wpool.tile([P, S], F32, tag="s23", bufs=1, name="fstageB")
        for hp in range(4):
            fvh = vhp.tile([P, 2, 4, DH + 1], BF, tag="vh", bufs=2,
                           name="fvh")
            nc.gpsimd.memset(fvh[:, :, :, DH:DH + 1], 1.0)
            for hi in range(2):
                nc.sync.dma_start(
                    out=fvh[:, hi, :, 0:DH],
                    in_=fvrows[:, :,
                               (2 * hp + hi) * DH:(2 * hp + hi + 1) * DH])
            ps_o0 = psacc.tile([DH + 1, S], F32, tag="acc", name="fps_o0")
            ps_o1 = psacc.tile([DH + 1, S], F32, tag="acc", name="fps_o1")
            for kt in range(4):
                ps_s = pssc.tile([P, 2 * S], F32, tag="sc", name="fps_s")
                mm(ps_s[:, 0:S],
                   fkT[0:DH, hp, kt * P:(kt + 1) * P],
                   fqTq[0:DH, hp, :], True, True)
                mm(ps_s[:, S:2 * S],
                   fkT[DH:P, hp, kt * P:(kt + 1) * P],
                   fqTq[DH:P, hp, :], True, True)
                et = expp.tile([P, 2 * S], BF, tag="expT", name="fet")
                nc.scalar.activation(out=et[:], in_=ps_s[:], func=AF.Exp,
                                     scale=SCALE)
                mm(ps_o0[:], fvh[:, 0, kt, :], et[:, 0:S], kt == 0, kt == 3)
                mm(ps_o1[:], fvh[:, 1, kt, :], et[:, S:2 * S], kt == 0,
                   kt == 3)
            nc.vector.tensor_copy(out=foTr[0:DH, hp, :], in_=ps_o0[0:DH, :])
            nc.vector.tensor_copy(out=foTr[DH:P, hp, :], in_=ps_o1[0:DH, :])
            nc.vector.tensor_copy(out=fstageA[32 * hp:32 * hp + 1, :],
                                  in_=ps_o0[DH:DH + 1, :])
            nc.vector.tensor_copy(out=fstageB[32 * hp:32 * hp + 1, :],
                                  in_=ps_o1[DH:DH + 1, :])
        fosum8 = persist.tile([H, S], F32, tag="osum8")
        for h in range(H):
            src = fstageA if h % 2 == 0 else fstageB
            nc.sync.dma_start(out=fosum8[h:h + 1, :],
                              in_=src[32 * (h // 2):32 * (h // 2) + 1, :])
        forec = persist.tile([H, S], F32, tag="orec")
        nc.vector.reciprocal(out=forec[:], in_=fosum8[:])
        foT = persist.tile([P, DCH, S], BF, tag="h1_bf")
        for ch in range(DCH):
            ps_r = pt([P, S], F32, "ps_rbc")
            mm(ps_r[:], ind_sb[:, ch * P:(ch + 1) * P], forec[:], True, True)
            nc.vector.tensor_mul(out=foT[:, ch, :], in0=foTr[:, ch, :],
                                 in1=ps_r[:])

        ft = persist.tile([P, DCH, S], F32, tag="slotC")
        for m in range(DCH):
            ps = pt([P, S], F32, "ps_mm")
            for kc in range(DCH):
                mm(ps[:], f_out_w_sb[:, kc, m * P:(m + 1) * P],
                   foT[:, kc, :], kc == 0, kc == DCH - 1)
            t1 = wpool.tile([P, S], F32, tag="oproj", bufs=1, name="foproj")
            nc.scalar.activation(out=t1[:], in_=ps[:], func=AF.Identity,
                                 bias=vpsl("f_out_b", m, m + 1))
            nc.vector.tensor_add(out=ft[:, m, :], in0=t1[:],
                                 in1=fqT_bf[:, m, :])
        ft_bf = persist.tile([P, DCH, S], BF, tag="h2_bf")
        layernorm_T(ft, "ans2_g", "ans2_b", ft_bf)

        fmidf = persist.tile([P, FMID2 // P, S], BF, tag="qT")
        for m in range(FMID2 // P):
            ps = pt([P, S], F32, "ps_mm")
            for kc in range(DCH):
                mm(ps[:], lrl_w1_sb[:, kc, m * P:(m + 1) * P],
                   ft_bf[:, kc, :], kc == 0, kc == DCH - 1)
            nc.scalar.activation(out=fmidf[:, m, :], in_=ps[:], func=AF.Relu,
                                 bias=vpsl("lrl_b1", m, m + 1))
        for m in range(AOUT // P):
            ps = pt([P, S], F32, "ps_mm")
            for kc in range(FMID2 // P):
                mm(ps[:], lrl_w2_sb[:, kc, m * P:(m + 1) * P],
                   fmidf[:, kc, :], kc == 0, kc == FMID2 // P - 1)
            fout = wpool.tile([P, S], F32, tag="aout", bufs=1, name="fout")
            nc.scalar.activation(out=fout[:], in_=ps[:], func=AF.Identity,
                                 bias=vpsl("lrl_b2", m, m + 1))
            nc.sync.dma_start(out=out_fastaT[m * P:(m + 1) * P, :],
                              in_=fout[:])

    nc.finalize()
    return nc


_NC_CACHE = {}
_LAST_IN_MAPS = None


def _get_nc():
    if "nc" not in _NC_CACHE:
        _NC_CACHE["nc"] = build_nc()
    return _NC_CACHE["nc"]


def _chunked_vec(v):
    """[n] f32 -> [128, n/128]: column c holds dims [128c, 128c+128)."""
    v = np.asarray(v, np.float32)
    return np.ascontiguousarray(v.reshape(-1, P).T)


def kernel(out_atom, out_fasta, query_list, centroids,
           a_in_w, a_in_b, a_out_w, a_out_b, an1_g, an1_b,
           fw1_w1, fw1_b1, fw1_w2, fw1_b2, an2_g, an2_b,
           fw2_w1, fw2_b1, fw2_w2, fw2_b2,
           f_in_w, f_in_b, f_out_w, f_out_b, ans2_g, ans2_b,
           lrl_w1, lrl_b1, lrl_w2, lrl_b2):
    out_atom = np.asarray(out_atom, np.float32)
    out_fasta = np.asarray(out_fasta, np.float32)
    query_list = np.asarray(query_list, np.int64)
    centroids = np.asarray(centroids, np.int32)

    # host-side repeat-interleave (mirrors jnp.repeat w/ total_repeat_length)
    idx = np.repeat(np.arange(NF), query_list)
    if idx.size < NA:
        idx = np.concatenate([idx, np.full(NA - idx.size, NF - 1, np.int64)])
    idx = idx[:NA]
    align_rows = out_fasta[0][idx]                      # [NA, D]

    # indicator matrix: head -> its 64 dims (for denominator broadcast)
    ind = np.zeros((H, D), np.float32)
    for h in range(H):
        ind[h, h * DH:(h + 1) * DH] = 1.0

    def bfc(a):
        return np.ascontiguousarray(np.asarray(a, np.float32).astype(bf16))

    vec_order = ["a_in_b_qk", "a_out_b", "an1_g", "an1_b", "fw1_b1",
                 "fw1_b2", "an2_g", "an2_b", "fw2_b1", "fw2_b2",
                 "f_in_b_qk", "f_out_b", "ans2_g", "ans2_b", "lrl_b1",
                 "lrl_b2"]
    vec_vals = {
        "a_in_b_qk": _chunked_vec(np.asarray(a_in_b)[:2 * D]),
        "a_out_b": _chunked_vec(a_out_b),
        "an1_g": _chunked_vec(an1_g), "an1_b": _chunked_vec(an1_b),
        "fw1_b1": _chunked_vec(fw1_b1), "fw1_b2": _chunked_vec(fw1_b2),
        "an2_g": _chunked_vec(an2_g), "an2_b": _chunked_vec(an2_b),
        "fw2_b1": _chunked_vec(fw2_b1), "fw2_b2": _chunked_vec(fw2_b2),
        "f_in_b_qk": _chunked_vec(np.asarray(f_in_b)[:2 * D]),
        "f_out_b": _chunked_vec(f_out_b),
        "ans2_g": _chunked_vec(ans2_g), "ans2_b": _chunked_vec(ans2_b),
        "lrl_b1": _chunked_vec(lrl_b1), "lrl_b2": _chunked_vec(lrl_b2),
    }
    vecpack = np.ascontiguousarray(
        np.concatenate([vec_vals[k] for k in vec_order], axis=1))
    vbias = np.ascontiguousarray(np.stack([
        np.asarray(a_in_b, np.float32)[2 * D:],
        np.asarray(f_in_b, np.float32)[2 * D:]]).astype(bf16))

    shared = {
        "a_in_w": bfc(a_in_w), "a_out_w": bfc(a_out_w),
        "fw1_w1": bfc(fw1_w1), "fw1_w2": bfc(fw1_w2),
        "fw2_w1": bfc(fw2_w1), "fw2_w2": bfc(fw2_w2),
        "f_in_w": bfc(f_in_w), "f_out_w": bfc(f_out_w),
        "lrl_w1": bfc(lrl_w1), "lrl_w2": bfc(lrl_w2),
        "vecpack": vecpack, "vbias": vbias,
        "x_fastaT_bf": np.ascontiguousarray(out_fasta[0].T.astype(bf16)),
        "ind8": ind,
    }

    in_maps = []
    for i in range(NCORES):
        sa, sf = slice(S * i, S * (i + 1)), slice(F * i, F * (i + 1))
        m = dict(shared)
        xs = (out_atom[0, sa] + align_rows[sa]).T
        m["x_sumT_bf"] = np.ascontiguousarray(xs.astype(bf16))
        m["centpk"] = np.ascontiguousarray(np.concatenate(
            [np.concatenate([centroids[0, sf], centroids[1, sf]], axis=0),
             np.concatenate([centroids[2, sf], centroids[3, sf]], axis=0)],
            axis=1))
        in_maps.append(m)

    global _LAST_IN_MAPS
    _LAST_IN_MAPS = in_maps
    nc = _get_nc()
    res = run_bass_kernel_spmd(nc, in_maps, core_ids=list(range(NCORES)))

    atom_out = np.concatenate(
        [res.results[i]["out_atomT"].T for i in range(NCORES)], axis=0)[None]
    fasta_out = res.results[0]["out_fastaT"].T[None]
    return (np.ascontiguousarray(atom_out),
            np.ascontiguousarray(fasta_out))


# revision 33
# speedup vs baseline: 1.1413x; 1.1413x over previous
"""Distributed Trainium2 Bass kernel for the AlignFormer block.

Sharding: sequence-parallel over 8 NeuronCores. Core i owns atom rows
[512*i, 512*(i+1)) and fasta rows [64*i, 64*(i+1)). Activations are kept in
transposed layout [D on partitions (4 chunks of 128), seq on free] so that
matmul chains need no transposes; layernorm statistics are computed with
ones-matmuls (partition-axis reduction on the TensorEngine).

Attention: scores are computed transposed (k on partitions, q on free), two
heads at a time (even head on PE rows 0-63, odd head on rows 64-127 so
LDWEIGHTS overlaps the other head's matmul); softmax skips the
max-subtraction (scores are O(5) for this problem, exp is exact) and the
denominator comes from a ones-augmented V column in the same PSUM
accumulation as att@V. Normalization is deferred: raw o and the per-head
denominators are collected, then one reciprocal + 4 indicator-matmuls
broadcast the scale factors.

The centroid gather runs against an all-gathered bf16 atom_trans; the fasta
branch is replicated on every core after a single AllGather of out_align.

Cross-core traffic: AllGather of K^T, V, atom_trans, out_align.
"""

from contextlib import ExitStack

import numpy as np
import ml_dtypes

import concourse.bass as bass
import concourse.mybir as mybir
from concourse import bacc, tile
from concourse.bass_utils import run_bass_kernel_spmd
from concourse.masks import make_identity

NCORES = 8
D = 512
NA = 4096
NF = 512
C = 4
KNB = 16
H = 8
DH = 64
FMID1 = 1024
FMID2 = 512
AOUT = 512
S = NA // NCORES   # 512 atom rows per core
F = NF // NCORES   # 64 fasta rows per core
P = 128
DCH = D // P       # 4 chunks of the model dim
BF = mybir.dt.bfloat16
F32 = mybir.dt.float32
I32 = mybir.dt.int32
SCALE = 1.0 / 8.0  # 1/sqrt(DH)
EPS = 1e-5
AF = mybir.ActivationFunctionType

bf16 = ml_dtypes.bfloat16


def build_nc():
    nc = bacc.Bacc("TRN2", target_bir_lowering=False, debug=False,
                   num_devices=NCORES)
    rg = [list(range(NCORES))]

    def mm(out, lhsT, rhs, start, stop):
        nc.tensor.matmul(out=out, lhsT=lhsT, rhs=rhs, start=start, stop=stop)

    # ---------------- external inputs ----------------
    x_sumT_bf = nc.dram_tensor("x_sumT_bf", [D, S], BF, kind="ExternalInput")
    x_fastaT_bf = nc.dram_tensor("x_fastaT_bf", [D, NF], BF,
                                 kind="ExternalInput")
    centpk = nc.dram_tensor("centpk", [P, 2 * KNB], I32, kind="ExternalInput")
    ind8 = nc.dram_tensor("ind8", [H, D], F32, kind="ExternalInput")

    a_in_w = nc.dram_tensor("a_in_w", [D, 3 * D], BF, kind="ExternalInput")
    a_out_w = nc.dram_tensor("a_out_w", [D, D], BF, kind="ExternalInput")
    fw1_w1 = nc.dram_tensor("fw1_w1", [D, FMID1], BF, kind="ExternalInput")
    fw1_w2 = nc.dram_tensor("fw1_w2", [FMID1, D], BF, kind="ExternalInput")
    fw2_w1 = nc.dram_tensor("fw2_w1", [D, FMID2], BF, kind="ExternalInput")
    fw2_w2 = nc.dram_tensor("fw2_w2", [FMID2, AOUT], BF, kind="ExternalInput")
    f_in_w = nc.dram_tensor("f_in_w", [D, 3 * D], BF, kind="ExternalInput")
    f_out_w = nc.dram_tensor("f_out_w", [D, D], BF, kind="ExternalInput")
    lrl_w1 = nc.dram_tensor("lrl_w1", [D, FMID2], BF, kind="ExternalInput")
    lrl_w2 = nc.dram_tensor("lrl_w2", [FMID2, AOUT], BF, kind="ExternalInput")

    # packed per-partition vectors ([128, nchunks])
    vec_specs = {
        "a_in_b_qk": 8, "a_out_b": 4, "an1_g": 4, "an1_b": 4,
        "fw1_b1": 8, "fw1_b2": 4, "an2_g": 4, "an2_b": 4,
        "fw2_b1": 4, "fw2_b2": 4, "f_in_b_qk": 8, "f_out_b": 4,
        "ans2_g": 4, "ans2_b": 4, "lrl_b1": 4, "lrl_b2": 4,
    }
    nvec = sum(vec_specs.values())
    vecpack = nc.dram_tensor("vecpack", [P, nvec], F32, kind="ExternalInput")
    vbias = nc.dram_tensor("vbias", [2, D], BF, kind="ExternalInput")

    out_atomT = nc.dram_tensor("out_atomT", [AOUT, S], F32,
                               kind="ExternalOutput")
    out_fastaT = nc.dram_tensor("out_fastaT", [AOUT, NF], F32,
                                kind="ExternalOutput")

    with tile.TileContext(nc) as tc, ExitStack() as ctx:
        dram = ctx.enter_context(tc.tile_pool(name="dram", bufs=1,
                                              space="DRAM"))
        const = ctx.enter_context(tc.tile_pool(name="const", bufs=1))
        persist = ctx.enter_context(tc.tile_pool(name="persist", bufs=1))
        wpool = ctx.enter_context(tc.tile_pool(name="wpool", bufs=2))
        expp = ctx.enter_context(tc.tile_pool(name="expp", bufs=3))
        vhp = ctx.enter_context(tc.tile_pool(name="vhp", bufs=4))
        psmm = ctx.enter_context(tc.tile_pool(name="psmm", bufs=2,
                                              space="PSUM"))
        pssc = ctx.enter_context(tc.tile_pool(name="pssc", bufs=2,
                                              space="PSUM"))
        psacc = ctx.enter_context(tc.tile_pool(name="psacc", bufs=2,
                                               space="PSUM"))

        def pt(shape, dtype, tag):
            return psmm.tile(shape, dtype, tag="mm", name=tag)

        # ---------------- DRAM bounce / collective buffers ----------------
        warm_in = dram.tile([1, P], BF)
        warm_out = dram.tile([NCORES, P], BF, addr_space="Shared")
        nc.gpsimd.collective_compute(
            "AllGather", mybir.AluOpType.bypass, ins=[warm_in[:]],
            outs=[warm_out[:]], replica_groups=rg)
        bounce_kv = dram.tile([2 * D, S], BF)
        kv_full = dram.tile([NCORES * 2 * D, S], BF, addr_space="Shared")
        bounce_atr = dram.tile([S, D], BF)
        atr_full = dram.tile([NA, D], BF, addr_space="Shared")
        bounce_oaa = dram.tile([F, D], F32)
        oaa_full = dram.tile([NF, D], F32, addr_space="Shared")

        # ---------------- constants ----------------
        ident = const.tile([P, P], F32)
        make_identity(nc, ident[:])
        ones_col = const.tile([P, 1], F32)
        nc.vector.memset(ones_col[:], 1.0)
        ones_row = const.tile([1, P], F32)
        nc.vector.memset(ones_row[:], 1.0)
        ones_row_bf = const.tile([1, P], BF)
        nc.vector.memset(ones_row_bf[:], 1.0)
        eps_tile = const.tile([1, 1], F32)
        nc.vector.memset(eps_tile[:], EPS)
        # warm the exp table early (one-time ~2.7us table load)
        dummy = const.tile([1, 1], F32)
        nc.scalar.activation(out=dummy[:], in_=ones_col[:1, :], func=AF.Exp)

        vp_sb = const.tile([P, nvec], F32)
        nc.sync.dma_start(out=vp_sb[:], in_=vecpack[:])
        VOFF = {}
        _off = 0
        for k, n in vec_specs.items():
            VOFF[k] = _off
            _off += n

        def vpsl(k, a, b):
            return vp_sb[:, VOFF[k] + (a):VOFF[k] + (b)]

        a_in_b_v_t = const.tile([1, D], BF)
        nc.sync.dma_start(out=a_in_b_v_t[:], in_=vbias[0:1, :])
        f_in_b_v_t = const.tile([1, D], BF)
        nc.sync.dma_start(out=f_in_b_v_t[:], in_=vbias[1:2, :])
        a_in_b_v_sb = a_in_b_v_t[:]
        f_in_b_v_sb = f_in_b_v_t[:]
        centpk_sb = const.tile([P, 2 * KNB], I32)
        nc.sync.dma_start(out=centpk_sb[:], in_=centpk[:])
        cent01_sb = centpk_sb[:, 0:KNB]
        cent23_sb = centpk_sb[:, KNB:2 * KNB]
        ind_sb = const.tile([H, D], F32)
        nc.sync.dma_start(out=ind_sb[:], in_=ind8[:])

        def load_w(ext, nch, width, tag):
            t = persist.tile([P, nch, width], BF, name=ext.name + "_sb",
                             tag=tag)
            nc.sync.dma_start(
                out=t[:], in_=ext[:].rearrange("(c p) n -> p c n", p=P))
            return t

        # only the QKV weight is needed before the collectives
        a_in_w_sb = load_w(a_in_w, DCH, 3 * D, "w_in")

        # ---------------- phase A: load pre-transposed atom_in ----------
        atom_inT_bf = persist.tile([P, DCH, S], BF, tag="bfA")
        nc.sync.dma_start(
            out=atom_inT_bf[:],
            in_=x_sumT_bf[:].rearrange("(c p) s -> p c s", p=P))

        # ---------------- phase B: QKV projections + allgathers ----------
        for m in range(DCH):  # K^T -> bounce -> AG (first, to overlap)
            ps = pt([P, S], F32, "ps_mm")
            for kc in range(DCH):
                mm(ps[:], a_in_w_sb[:, kc, D + m * P:D + (m + 1) * P],
                   atom_inT_bf[:, kc, :], kc == 0, kc == DCH - 1)
            stg = wpool.tile([P, S], BF, tag="stage", name="stg")
            nc.vector.tensor_scalar_add(
                out=stg[:], in0=ps[:], scalar1=vpsl("a_in_b_qk", 4 + m, 5 + m))
            nc.sync.dma_start(out=bounce_kv[m * P:(m + 1) * P, :], in_=stg[:])

        for st in range(4):  # V rows -> bounce -> AG
            ps = pt([P, D], F32, "ps_mm")
            for kc in range(DCH):
                mm(ps[:], atom_inT_bf[:, kc, st * P:(st + 1) * P],
                   a_in_w_sb[:, kc, 2 * D:3 * D], kc == 0, False)
            mm(ps[:], ones_row_bf[:], a_in_b_v_sb, False, True)
            stg = wpool.tile([P, S], BF, tag="stage", name="stg")
            nc.vector.tensor_copy(out=stg[:], in_=ps[:])
            nc.sync.dma_start(out=bounce_kv[D + st * P:D + (st + 1) * P, :],
                              in_=stg[:])
        nc.gpsimd.collective_compute(
            "AllGather", mybir.AluOpType.bypass, ins=[bounce_kv[:]],
            outs=[kv_full[:]], replica_groups=rg)

        qT = persist.tile([P, DCH, S], BF)   # Q^T stays local
        for m in range(DCH):
            ps = pt([P, S], F32, "ps_mm")
            for kc in range(DCH):
                mm(ps[:], a_in_w_sb[:, kc, m * P:(m + 1) * P],
                   atom_inT_bf[:, kc, :], kc == 0, kc == DCH - 1)
            nc.vector.tensor_scalar_add(
                out=qT[:, m, :], in0=ps[:],
                scalar1=vpsl("a_in_b_qk", m, m + 1))

        # ---------------- fasta pre-work (independent of the gather) -----
        f_in_w_sb = load_w(f_in_w, DCH, 3 * D, "w_in")
        fxT_bf = persist.tile([P, DCH, S], BF, tag="fxbf")
        nc.sync.dma_start(
            out=fxT_bf[:],
            in_=x_fastaT_bf[:].rearrange("(c p) s -> p c s", p=P))
        k1T = persist.tile([P, DCH, S], BF, tag="k1T")
        for m in range(DCH):
            ps = pt([P, S], F32, "ps_mm")
            for kc in range(DCH):
                mm(ps[:], f_in_w_sb[:, kc, D + m * P:D + (m + 1) * P],
                   fxT_bf[:, kc, :], kc == 0, kc == DCH - 1)
            nc.vector.tensor_scalar_add(
                out=k1T[:, m, :], in0=ps[:],
                scalar1=vpsl("f_in_b_qk", 4 + m, 5 + m))
        q1T = persist.tile([P, DCH, S], BF, tag="q1T")
        for m in range(DCH):
            ps = pt([P, S], F32, "ps_mm")
            for kc in range(DCH):
                mm(ps[:], f_in_w_sb[:, kc, m * P:(m + 1) * P],
                   fxT_bf[:, kc, :], kc == 0, kc == DCH - 1)
            nc.vector.tensor_scalar_add(
                out=q1T[:, m, :], in0=ps[:],
                scalar1=vpsl("f_in_b_qk", m, m + 1))
        v1r = persist.tile([P, 4, D], BF, tag="v1r")
        for st in range(4):
            ps = pt([P, D], F32, "ps_mm")
            for kc in range(DCH):
                mm(ps[:], fxT_bf[:, kc, st * P:(st + 1) * P],
                   f_in_w_sb[:, kc, 2 * D:3 * D], kc == 0, False)
            mm(ps[:], ones_row_bf[:], f_in_b_v_sb, False, True)
            nc.vector.tensor_copy(out=v1r[:, st, :], in_=ps[:])

        # remaining weights (loads overlap the collectives / attention)
        a_out_w_sb = load_w(a_out_w, DCH, D, "w_proj")
        fw1_w1_sb = load_w(fw1_w1, DCH, FMID1, "w_big1")
        fw1_w2_sb = load_w(fw1_w2, FMID1 // P, D, "w_big2")
        fw2_w1_sb = load_w(fw2_w1, DCH, FMID2, "w_fw2a")
        fw2_w2_sb = load_w(fw2_w2, FMID2 // P, AOUT, "w_fw2b")

        # ---------------- phase C/D: attention (head pairs) --------------
        ktb = []
        for b in range(NCORES):
            kt_t = persist.tile([P, DCH, S], BF, name=f"ktb{b}",
                                tag=f"ktb{b}")
            nc.sync.dma_start(
                out=kt_t[:],
                in_=kv_full[b * 2 * D:b * 2 * D + D, :].rearrange(
                    "(c p) s -> p c s", p=P))
            ktb.append(kt_t)
        v_full_r = kv_full[:].rearrange(
            "(b t st p) (hh j) -> t p b st hh j", t=2, st=4, p=P, j=DH)

        oTr = persist.tile([P, DCH, S], BF)       # raw (unnormalized) o^T
        stageA = wpool.tile([P, S], F32, tag="s01", bufs=1, name="stageA")
        stageB = wpool.tile([P, S], F32, tag="s23", bufs=1, name="stageB")
        for hp in range(4):
            vh0 = vhp.tile([P, 8, 4, DH + 1], BF, tag="vh", name="vh0")
            nc.gpsimd.memset(vh0[:, :, :, DH:DH + 1], 1.0)
            for b in range(NCORES):
                nc.sync.dma_start(out=vh0[:, b, :, 0:DH],
                                  in_=v_full_r[1, :, b, :, 2 * hp, :])
            vh1 = vhp.tile([P, 8, 4, DH + 1], BF, tag="vh", name="vh1")
            nc.gpsimd.memset(vh1[:, :, :, DH:DH + 1], 1.0)
            for b in range(NCORES):
                nc.sync.dma_start(out=vh1[:, b, :, 0:DH],
                                  in_=v_full_r[1, :, b, :, 2 * hp + 1, :])
            ps_o0 = psacc.tile([DH + 1, S], F32, tag="acc", name="ps_o0")
            ps_o1 = psacc.tile([DH + 1, S], F32, tag="acc", name="ps_o1")
            for kt in range(32):
                b, ktl = kt // 4, kt % 4
                ps_s = pssc.tile([P, 2 * S], F32, tag="sc", name="ps_s")
                mm(ps_s[:, 0:S],
                   ktb[b][0:DH, hp, ktl * P:(ktl + 1) * P],
                   qT[0:DH, hp, :], True, True)
                mm(ps_s[:, S:2 * S],
                   ktb[b][DH:P, hp, ktl * P:(ktl + 1) * P],
                   qT[DH:P, hp, :], True, True)
                et = expp.tile([P, 2 * S], BF, tag="expT", name="et")
                nc.scalar.activation(out=et[:], in_=ps_s[:], func=AF.Exp,
                                     scale=SCALE)
                mm(ps_o0[:], vh0[:, b, ktl, :], et[:, 0:S], kt == 0, kt == 31)
                mm(ps_o1[:], vh1[:, b, ktl, :], et[:, S:2 * S], kt == 0,
                   kt == 31)
            nc.vector.tensor_copy(out=oTr[0:DH, hp, :], in_=ps_o0[0:DH, :])
            nc.vector.tensor_copy(out=oTr[DH:P, hp, :], in_=ps_o1[0:DH, :])
            nc.vector.tensor_copy(out=stageA[32 * hp:32 * hp + 1, :],
                                  in_=ps_o0[DH:DH + 1, :])
            nc.vector.tensor_copy(out=stageB[32 * hp:32 * hp + 1, :],
                                  in_=ps_o1[DH:DH + 1, :])
        # deferred softmax normalization: oT = oTr * (1/osum) per (head, q)
        osum8 = persist.tile([H, S], F32, tag="osum8")
        for h in range(H):
            src = stageA if h % 2 == 0 else stageB
            nc.sync.dma_start(out=osum8[h:h + 1, :],
                              in_=src[32 * (h // 2):32 * (h // 2) + 1, :])
        orec = persist.tile([H, S], F32, tag="orec")
        nc.vector.reciprocal(out=orec[:], in_=osum8[:])
        oT = persist.tile([P, DCH, S], BF)
        for ch in range(DCH):
            ps_r = pt([P, S], F32, "ps_rbc")
            mm(ps_r[:], ind_sb[:, ch * P:(ch + 1) * P], orec[:], True, True)
            nc.vector.tensor_mul(out=oT[:, ch, :], in0=oTr[:, ch, :],
                                 in1=ps_r[:])

        # ---------------- layernorm helper (transposed layout) -----------
        def layernorm_T(pre, gk, bk, out_bf, out_f32=None):
            Sw = pre.shape[2]
            ps_mean = pt([1, Sw], F32, "ps_mean")
            ps_var = pt([1, Sw], F32, "ps_var")
            for m in range(DCH):
                mm(ps_mean[:], ones_col[:], pre[:, m, :], m == 0, m == DCH - 1)
                sq = wpool.tile([P, Sw], F32, tag="sq", bufs=1, name="sq")
                nc.scalar.activation(out=sq[:], in_=pre[:, m, :],
                                     func=AF.Square)
                mm(ps_var[:], ones_col[:], sq[:], m == 0, m == DCH - 1)
            mvec = wpool.tile([1, Sw], F32, tag="lnvec", bufs=3, name="mvec")
            nc.scalar.mul(out=mvec[:], in_=ps_mean[:], mul=1.0 / D)
            vvec = wpool.tile([1, Sw], F32, tag="lnvec", bufs=3, name="vvec")
            nc.scalar.mul(out=vvec[:], in_=ps_var[:], mul=1.0 / D)
            msq = wpool.tile([1, Sw], F32, tag="lnvec", bufs=3, name="msq")
            nc.vector.tensor_mul(out=msq[:], in0=mvec[:], in1=mvec[:])
            nc.vector.tensor_sub(out=vvec[:], in0=vvec[:], in1=msq[:])
            sd = wpool.tile([1, Sw], F32, tag="lnvec", bufs=3, name="sd")
            nc.scalar.activation(out=sd[:], in_=vvec[:], func=AF.Sqrt,
                                 bias=eps_tile[:])
            rstd = wpool.tile([1, Sw], F32, tag="lnvec", bufs=3, name="rstd")
            nc.vector.reciprocal(out=rstd[:], in_=sd[:])
            ps_mb = pt([P, Sw], F32, "ps_mb")
            mm(ps_mb[:], ones_row[:], mvec[:], True, True)
            ps_rb = pt([P, Sw], F32, "ps_rb")
            mm(ps_rb[:], ones_row[:], rstd[:], True, True)
            for m in range(DCH):
                t1 = wpool.tile([P, Sw], F32, tag="lnt1", bufs=1, name="lnt1")
                nc.vector.tensor_sub(out=t1[:], in0=pre[:, m, :], in1=ps_mb[:])
                nc.vector.tensor_mul(out=t1[:], in0=t1[:], in1=ps_rb[:])
                if out_f32 is not None:
                    nc.vector.tensor_scalar(
                        out=out_f32[:, m, :], in0=t1[:],
                        scalar1=vpsl(gk, m, m + 1), scalar2=vpsl(bk, m, m + 1),
                        op0=mybir.AluOpType.mult, op1=mybir.AluOpType.add)
                    nc.scalar.copy(out=out_bf[:, m, :], in_=out_f32[:, m, :])
                else:
                    nc.vector.tensor_scalar(
                        out=out_bf[:, m, :], in0=t1[:],
                        scalar1=vpsl(gk, m, m + 1), scalar2=vpsl(bk, m, m + 1),
                        op0=mybir.AluOpType.mult, op1=mybir.AluOpType.add)

        # ---------------- phase E: out-proj + residual + LN1 -------------
        h1 = persist.tile([P, DCH, S], F32, tag="h1")
        for m in range(DCH):
            ps = pt([P, S], F32, "ps_mm")
            for kc in range(DCH):
                mm(ps[:], a_out_w_sb[:, kc, m * P:(m + 1) * P],
                   oT[:, kc, :], kc == 0, kc == DCH - 1)
            t1 = wpool.tile([P, S], F32, tag="oproj", bufs=1, name="oproj")
            nc.scalar.activation(out=t1[:], in_=ps[:], func=AF.Identity,
                                 bias=vpsl("a_out_b", m, m + 1))
            nc.vector.tensor_add(out=h1[:, m, :], in0=t1[:],
                                 in1=atom_inT_bf[:, m, :])
        h1_bf = persist.tile([P, DCH, S], BF, tag="h1_bf")
        layernorm_T(h1, "an1_g", "an1_b", h1_bf, h1)

        # ---------------- phase F: FFN1 + atom_trans allgather -----------
        fmid = persist.tile([P, FMID1 // P, S], BF, tag="slotA")
        for m in range(FMID1 // P):
            ps = pt([P, S], F32, "ps_mm")
            for kc in range(DCH):
                mm(ps[:], fw1_w1_sb[:, kc, m * P:(m + 1) * P],
                   h1_bf[:, kc, :], kc == 0, kc == DCH - 1)
            nc.scalar.activation(out=fmid[:, m, :], in_=ps[:], func=AF.Relu,
                                 bias=vpsl("fw1_b1", m, m + 1))
        atrT = persist.tile([P, DCH, S], F32, tag="slotC")
        for m in range(DCH):
            ps = pt([P, S], F32, "ps_mm")
            for kc in range(FMID1 // P):
                mm(ps[:], fw1_w2_sb[:, kc, m * P:(m + 1) * P],
                   fmid[:, kc, :], kc == 0, kc == FMID1 // P - 1)
            nc.scalar.activation(out=atrT[:, m, :], in_=ps[:],
                                 func=AF.Identity,
                                 bias=vpsl("fw1_b2", m, m + 1))
        for st in range(4):  # back to rows (bf16) for the gather allgather
            arow = wpool.tile([P, D], BF, tag="arow", bufs=1, name="arow")
            for dc in range(DCH):
                pst = pt([P, P], F32, "ps_tr")
                nc.tensor.transpose(
                    out=pst[:], in_=atrT[:, dc, st * P:(st + 1) * P],
                    identity=ident[:])
                nc.vector.tensor_copy(out=arow[:, dc * P:(dc + 1) * P],
                                      in_=pst[:])
            nc.sync.dma_start(out=bounce_atr[st * P:(st + 1) * P, :],
                              in_=arow[:])
        nc.gpsimd.collective_compute(
            "AllGather", mybir.AluOpType.bypass, ins=[bounce_atr[:]],
            outs=[atr_full[:]], replica_groups=rg)

        # ---------------- phase G: LN2 + FFN2 -> atom_out ----------------
        h2pre = persist.tile([P, DCH, S], F32, tag="h2pre")
        for m in range(DCH):
            nc.vector.tensor_add(out=h2pre[:, m, :], in0=h1[:, m, :],
                                 in1=atrT[:, m, :])
        h2_bf = persist.tile([P, DCH, S], BF, tag="h2_bf")
        layernorm_T(h2pre, "an2_g", "an2_b", h2_bf)
        fmid2 = persist.tile([P, FMID2 // P, S], BF, tag="bfA")
        for m in range(FMID2 // P):
            ps = pt([P, S], F32, "ps_mm")
            for kc in range(DCH):
                mm(ps[:], fw2_w1_sb[:, kc, m * P:(m + 1) * P],
                   h2_bf[:, kc, :], kc == 0, kc == DCH - 1)
            nc.scalar.activation(out=fmid2[:, m, :], in_=ps[:], func=AF.Relu,
                                 bias=vpsl("fw2_b1", m, m + 1))
        for m in range(AOUT // P):
            ps = pt([P, S], F32, "ps_mm")
            for kc in range(FMID2 // P):
                mm(ps[:], fw2_w2_sb[:, kc, m * P:(m + 1) * P],
                   fmid2[:, kc, :], kc == 0, kc == FMID2 // P - 1)
            aout = wpool.tile([P, S], F32, tag="aout", bufs=1, name="aout")
            nc.scalar.activation(out=aout[:], in_=ps[:], func=AF.Identity,
                                 bias=vpsl("fw2_b2", m, m + 1))
            nc.sync.dma_start(out=out_atomT[m * P:(m + 1) * P, :], in_=aout[:])

        # ---------------- phase H: centroid gather + DVE max-accum ------
        gath0 = persist.tile([P, D], BF, tag="gath0")
        gath1 = persist.tile([P, D], BF, tag="gath1")
        for (gt, ct) in ((gath0, cent01_sb), (gath1, cent23_sb)):
            nc.vector.memset(gt[:], -3.0e38)
            for k in range(KNB):
                gb = wpool.tile([P, D], BF, tag="gbuf", bufs=2, name="gb")
                nc.gpsimd.indirect_dma_start(
                    out=gb[:], out_offset=None, in_=atr_full[:],
                    in_offset=bass.IndirectOffsetOnAxis(ap=ct[:, k:k + 1],
                                                        axis=0))
                nc.vector.tensor_tensor(out=gt[:], in0=gt[:], in1=gb[:],
                                        op=mybir.AluOpType.max)
        oaa = persist.tile([F, D], F32)
        s01 = wpool.tile([F, D], F32, tag="s01", bufs=1, name="s01")
        nc.vector.tensor_copy(out=s01[:], in_=gath0[F:P, :])
        nc.vector.tensor_add(out=s01[:], in0=s01[:], in1=gath0[0:F, :])
        s23 = wpool.tile([F, D], F32, tag="s23", bufs=1, name="s23")
        nc.vector.tensor_copy(out=s23[:], in_=gath1[F:P, :])
        nc.vector.tensor_add(out=s23[:], in0=s23[:], in1=gath1[0:F, :])
        nc.vector.tensor_add(out=oaa[:], in0=s01[:], in1=s23[:])
        nc.sync.dma_start(out=bounce_oaa[:], in_=oaa[:])
        nc.gpsimd.collective_compute(
            "AllGather", mybir.AluOpType.bypass, ins=[bounce_oaa[:]],
            outs=[oaa_full[:]], replica_groups=rg)

        # ---------------- phase I: fasta branch (replicated) -------------
        f_out_w_sb = load_w(f_out_w, DCH, D, "w_proj")
        lrl_w1_sb = load_w(lrl_w1, DCH, FMID2, "w_big1")
        lrl_w2_sb = load_w(lrl_w2, FMID2 // P, AOUT, "w_big2")
        # oaa part of fq: transpose gathered out_align and finish QKV
        oaaT_bf = persist.tile([P, DCH, S], BF, tag="oTr")
        for st in range(4):
            wa = wpool.tile([P, D], F32, tag="ld", bufs=2, name="wa")
            nc.sync.dma_start(out=wa[:],
                              in_=oaa_full[st * P:(st + 1) * P, :])
            for dc in range(DCH):
                pst = pt([P, P], F32, "ps_tr")
                nc.tensor.transpose(out=pst[:],
                                    in_=wa[:, dc * P:(dc + 1) * P],
                                    identity=ident[:])
                nc.scalar.copy(
                    out=oaaT_bf[:, dc, st * P:(st + 1) * P], in_=pst[:])
        fqT_bf = persist.tile([P, DCH, S], BF, tag="h2pre")
        nc.vector.tensor_add(out=fqT_bf[:], in0=fxT_bf[:], in1=oaaT_bf[:])
        for m in range(DCH):
            ps = pt([P, S], F32, "ps_mm")
            for kc in range(DCH):
                mm(ps[:], f_in_w_sb[:, kc, D + m * P:D + (m + 1) * P],
                   oaaT_bf[:, kc, :], kc == 0, kc == DCH - 1)
            nc.vector.tensor_add(out=k1T[:, m, :], in0=ps[:],
                                 in1=k1T[:, m, :])
        fkT = k1T
        for st in range(4):
            ps = pt([P, D], F32, "ps_mm")
            for kc in range(DCH):
                mm(ps[:], oaaT_bf[:, kc, st * P:(st + 1) * P],
                   f_in_w_sb[:, kc, 2 * D:3 * D], kc == 0, kc == DCH - 1)
            nc.vector.tensor_add(out=v1r[:, st, :], in0=ps[:],
                                 in1=v1r[:, st, :])
        fvrows = v1r
        for m in range(DCH):
            ps = pt([P, S], F32, "ps_mm")
            for kc in range(DCH):
                mm(ps[:], f_in_w_sb[:, kc, m * P:(m + 1) * P],
                   oaaT_bf[:, kc, :], kc == 0, kc == DCH - 1)
            nc.vector.tensor_add(out=q1T[:, m, :], in0=ps[:],
                                 in1=q1T[:, m, :])
        fqTq = q1T

        foTr = persist.tile([P, DCH, S], BF, tag="oTr")
        fstageA = wpool.tile([P, S], F32, tag="s01", bufs=1, name="fstageA")
        fstageB = wpool.tile([P, S], F32, tag="s23", bufs=1, name="fstageB")
        for hp in range(4):
            fvh = vhp.tile([P, 2, 4, DH + 1], BF, tag="vh", bufs=4,
                           name="fvh")
            nc.gpsimd.memset(fvh[:, :, :, DH:DH + 1], 1.0)
            for hi in range(2):
                nc.sync.dma_start(
                    out=fvh[:, hi, :, 0:DH],
                    in_=fvrows[:, :,
                               (2 * hp + hi) * DH:(2 * hp + hi + 1) * DH])
            ps_o0 = psacc.tile([DH + 1, S], F32, tag="acc", name="fps_o0")
            ps_o1 = psacc.tile([DH + 1, S], F32, tag="acc", name="fps_o1")
            for kt in range(4):
                ps_s = pssc.tile([P, 2 * S], F32, tag="sc", name="fps_s")
                mm(ps_s[:, 0:S],
                   fkT[0:DH, hp, kt * P:(kt + 1) * P],
                   fqTq[0:DH, hp, :], True, True)
                mm(ps_s[:, S:2 * S],
                   fkT[DH:P, hp, kt * P:(kt + 1) * P],
                   fqTq[DH:P, hp, :], True, True)
                et = expp.tile([P, 2 * S], BF, tag="expT", name="fet")
                nc.scalar.activation(out=et[:], in_=ps_s[:], func=AF.Exp,
                                     scale=SCALE)
                mm(ps_o0[:], fvh[:, 0, kt, :], et[:, 0:S], kt == 0, kt == 3)
                mm(ps_o1[:], fvh[:, 1, kt, :], et[:, S:2 * S], kt == 0,
                   kt == 3)
            nc.vector.tensor_copy(out=foTr[0:DH, hp, :], in_=ps_o0[0:DH, :])
            nc.vector.tensor_copy(out=foTr[DH:P, hp, :], in_=ps_o1[0:DH, :])
            nc.vector.tensor_copy(out=fstageA[32 * hp:32 * hp + 1, :],
                                  in_=ps_o0[DH:DH + 1, :])
            nc.vector.tensor_copy(out=fstageB[32 * hp:32 * hp + 1, :],
                                  in_=ps_o1[DH:DH + 1, :])
        fosum8 = persist.tile([H, S], F32, tag="osum8")
        for h in range(H):
            src = fstageA if h % 2 == 0 else fstageB
            nc.sync.dma_start(out=fosum8[h:h + 1, :],
                              in_=src[32 * (h // 2):32 * (h // 2) + 1, :])
        forec = persist.tile([H, S], F32, tag="orec")
        nc.vector.reciprocal(out=forec[:], in_=fosum8[:])
        foT = persist.tile([P, DCH, S], BF, tag="h1_bf")
        for ch in range(DCH):
            ps_r = pt([P, S], F32, "ps_rbc")
            mm(ps_r[:], ind_sb[:, ch * P:(ch + 1) * P], forec[:], True, True)
            nc.vector.tensor_mul(out=foT[:, ch, :], in0=foTr[:, ch, :],
                                 in1=ps_r[:])

        ft = persist.tile([P, DCH, S], F32, tag="slotC")
        for m in range(DCH):
            ps = pt([P, S], F32, "ps_mm")
            for kc in range(DCH):
                mm(ps[:], f_out_w_sb[:, kc, m * P:(m + 1) * P],
                   foT[:, kc, :], kc == 0, kc == DCH - 1)
            t1 = wpool.tile([P, S], F32, tag="oproj", bufs=1, name="foproj")
            nc.scalar.activation(out=t1[:], in_=ps[:], func=AF.Identity,
                                 bias=vpsl("f_out_b", m, m + 1))
            nc.vector.tensor_add(out=ft[:, m, :], in0=t1[:],
                                 in1=fqT_bf[:, m, :])
        ft_bf = persist.tile([P, DCH, S], BF, tag="h2_bf")
        layernorm_T(ft, "ans2_g", "ans2_b", ft_bf)

        fmidf = persist.tile([P, FMID2 // P, S], BF, tag="qT")
        for m in range(FMID2 // P):
            ps = pt([P, S], F32, "ps_mm")
            for kc in range(DCH):
                mm(ps[:], lrl_w1_sb[:, kc, m * P:(m + 1) * P],
                   ft_bf[:, kc, :], kc == 0, kc == DCH - 1)
            nc.scalar.activation(out=fmidf[:, m, :], in_=ps[:], func=AF.Relu,
                                 bias=vpsl("lrl_b1", m, m + 1))
        for m in range(AOUT // P):
            ps = pt([P, S], F32, "ps_mm")
            for kc in range(FMID2 // P):
                mm(ps[:], lrl_w2_sb[:, kc, m * P:(m + 1) * P],
                   fmidf[:, kc, :], kc == 0, kc == FMID2 // P - 1)
            fout = wpool.tile([P, S], F32, tag="aout", bufs=1, name="fout")
            nc.scalar.activation(out=fout[:], in_=ps[:], func=AF.Identity,
                                 bias=vpsl("lrl_b2", m, m + 1))
            nc.sync.dma_start(out=out_fastaT[m * P:(m + 1) * P, :],
                              in_=fout[:])

    nc.finalize()
    return nc


_NC_CACHE = {}
_LAST_IN_MAPS = None


def _get_nc():
    if "nc" not in _NC_CACHE:
        _NC_CACHE["nc"] = build_nc()
    return _NC_CACHE["nc"]


def _chunked_vec(v):
    """[n] f32 -> [128, n/128]: column c holds dims [128c, 128c+128)."""
    v = np.asarray(v, np.float32)
    return np.ascontiguousarray(v.reshape(-1, P).T)


def kernel(out_atom, out_fasta, query_list, centroids,
           a_in_w, a_in_b, a_out_w, a_out_b, an1_g, an1_b,
           fw1_w1, fw1_b1, fw1_w2, fw1_b2, an2_g, an2_b,
           fw2_w1, fw2_b1, fw2_w2, fw2_b2,
           f_in_w, f_in_b, f_out_w, f_out_b, ans2_g, ans2_b,
           lrl_w1, lrl_b1, lrl_w2, lrl_b2):
    out_atom = np.asarray(out_atom, np.float32)
    out_fasta = np.asarray(out_fasta, np.float32)
    query_list = np.asarray(query_list, np.int64)
    centroids = np.asarray(centroids, np.int32)

    # host-side repeat-interleave (mirrors jnp.repeat w/ total_repeat_length)
    idx = np.repeat(np.arange(NF), query_list)
    if idx.size < NA:
        idx = np.concatenate([idx, np.full(NA - idx.size, NF - 1, np.int64)])
    idx = idx[:NA]
    align_rows = out_fasta[0][idx]                      # [NA, D]

    # indicator matrix: head -> its 64 dims (for denominator broadcast)
    ind = np.zeros((H, D), np.float32)
    for h in range(H):
        ind[h, h * DH:(h + 1) * DH] = 1.0

    def bfc(a):
        return np.ascontiguousarray(np.asarray(a, np.float32).astype(bf16))

    vec_order = ["a_in_b_qk", "a_out_b", "an1_g", "an1_b", "fw1_b1",
                 "fw1_b2", "an2_g", "an2_b", "fw2_b1", "fw2_b2",
                 "f_in_b_qk", "f_out_b", "ans2_g", "ans2_b", "lrl_b1",
                 "lrl_b2"]
    vec_vals = {
        "a_in_b_qk": _chunked_vec(np.asarray(a_in_b)[:2 * D]),
        "a_out_b": _chunked_vec(a_out_b),
        "an1_g": _chunked_vec(an1_g), "an1_b": _chunked_vec(an1_b),
        "fw1_b1": _chunked_vec(fw1_b1), "fw1_b2": _chunked_vec(fw1_b2),
        "an2_g": _chunked_vec(an2_g), "an2_b": _chunked_vec(an2_b),
        "fw2_b1": _chunked_vec(fw2_b1), "fw2_b2": _chunked_vec(fw2_b2),
        "f_in_b_qk": _chunked_vec(np.asarray(f_in_b)[:2 * D]),
        "f_out_b": _chunked_vec(f_out_b),
        "ans2_g": _chunked_vec(ans2_g), "ans2_b": _chunked_vec(ans2_b),
        "lrl_b1": _chunked_vec(lrl_b1), "lrl_b2": _chunked_vec(lrl_b2),
    }
    vecpack = np.ascontiguousarray(
        np.concatenate([vec_vals[k] for k in vec_order], axis=1))
    vbias = np.ascontiguousarray(np.stack([
        np.asarray(a_in_b, np.float32)[2 * D:],
        np.asarray(f_in_b, np.float32)[2 * D:]]).astype(bf16))

    shared = {
        "a_in_w": bfc(a_in_w), "a_out_w": bfc(a_out_w),
        "fw1_w1": bfc(fw1_w1), "fw1_w2": bfc(fw1_w2),
        "fw2_w1": bfc(fw2_w1), "fw2_w2": bfc(fw2_w2),
        "f_in_w": bfc(f_in_w), "f_out_w": bfc(f_out_w),
        "lrl_w1": bfc(lrl_w1), "lrl_w2": bfc(lrl_w2),
        "vecpack": vecpack, "vbias": vbias,
        "x_fastaT_bf": np.ascontiguousarray(out_fasta[0].T.astype(bf16)),
        "ind8": ind,
    }

    in_maps = []
    for i in range(NCORES):
        sa, sf = slice(S * i, S * (i + 1)), slice(F * i, F * (i + 1))
        m = dict(shared)
        xs = (out_atom[0, sa] + align_rows[sa]).T
        m["x_sumT_bf"] = np.ascontiguousarray(xs.astype(bf16))
        m["centpk"] = np.ascontiguousarray(np.concatenate(
            [np.concatenate([centroids[0, sf], centroids[1, sf]], axis=0),
             np.concatenate([centroids[2, sf], centroids[3, sf]], axis=0)],
            axis=1))
        in_maps.append(m)

    global _LAST_IN_MAPS
    _LAST_IN_MAPS = in_maps
    nc = _get_nc()
    res = run_bass_kernel_spmd(nc, in_maps, core_ids=list(range(NCORES)))

    atom_out = np.concatenate(
        [res.results[i]["out_atomT"].T for i in range(NCORES)], axis=0)[None]
    fasta_out = res.results[0]["out_fastaT"].T[None]
    return (np.ascontiguousarray(atom_out),
            np.ascontiguousarray(fasta_out))
